# revision 1
# baseline (speedup 1.0000x reference)
"""Multi-head attention layer (B=2, S=2048, Dm=1024, H=16, dk=dv=64) on 8 TRN2
NeuronCores.

Sharding: core c = b*4 + g handles batch b and head group g (4 heads).
Inside each core everything is computed in a "transposed" dataflow so that no
on-device transposes are ever needed:

  qT/kT [d, s]   <- Wg.T @ X.T          (X.T supplied by host)
  v'    [s, d+1] <- X @ Wv_g, plus a ones column per head
  sT    [sk, sq] <- k . q               (scores, transposed orientation)
  eT    [sk, sq] <- exp(sT * scale)     (scale = 1/sqrt(B), reference quirk)
  oT+sum [65, sq] <- v'.T @ eT          (row 64 = softmax denominators)
  o     [dv, sq] <- oT * (1/sum)        (broadcast via tiny PE matmul)
  out   [s, dm]  <- sum_h o_h.T @ Wo_h  (partial; host sums over head groups)

Host folds bv and bo exactly: out += bv @ Wo + bo (softmax rows sum to 1).
"""

import numpy as np

_B, _S, _DM = 2, 2048, 1024
_H, _DK = 16, 64
_GROUPS = 4
_HC = _H // _GROUPS          # heads per core
_DG = _HC * _DK              # 256 projection cols per core
_P = 128
_SC = 512                    # matmul free-dim tile (one psum bank of fp32)
_SC2 = 1024                  # attention sq chunk (2 banks; exp batched over it)
_SCALE = float(1.0 / np.sqrt(2.0))  # reference scales by sqrt(batch), not dk

# "fp32r" = full fp32 data, float32r matmul mode (1 cyc/row when free>=256).
#   NOTE: walrus's BIR verifier requires f32r consumers to read f32r-rounded
#   producers, which plain-f32 DMA loads are not — so fp32r needs every
#   producer plumbed as f32r. bf16 is the supported fast path.
# "bf16"  = inputs/weights cast to bf16 on host, bf16 matmuls
MM_MODE = "bf16"
PROFILE = False
LAST_EXEC_NS = {"ns": None, "result": None}

_CACHE = {}


def _emit(nc, tc, io, mm_mode):
    from contextlib import ExitStack
    import concourse.mybir as mybir

    FP = mybir.dt.float32
    FPR = mybir.dt.float32r
    BF = mybir.dt.bfloat16
    AF = mybir.ActivationFunctionType

    XDT = FP if mm_mode == "fp32r" else BF   # dtype of X / W dram + sbuf
    EXPDT = BF                               # dtype of exp(att) and v'
    R = (lambda ap: ap.bitcast(FPR)) if mm_mode == "fp32r" else (lambda ap: ap)

    P, S, DM, DG, HC, DK, SC, SC2 = _P, _S, _DM, _DG, _HC, _DK, _SC, _SC2
    NM = DM // P    # 8 contraction chunks for projections
    ND = DG // P    # 2 d-tiles (2 heads each)
    NST = S // P    # 16 sk tiles
    NC2 = S // SC2  # 2 attention chunks

    ctx = ExitStack()
    with ctx:
        wp = ctx.enter_context(tc.tile_pool(name="w", bufs=1))
        wW = ctx.enter_context(tc.tile_pool(name="wW", bufs=2))
        # xs: one projection accumulation group keeps 8 staging tiles live;
        # 16 slots = current set + prefetched next set (keeps the DMA rings
        # from head-of-line blocking on slot waits)
        xs = ctx.enter_context(tc.tile_pool(name="xs", bufs=16))
        xv = ctx.enter_context(tc.tile_pool(name="xv", bufs=16))
        qk = ctx.enter_context(tc.tile_pool(name="qk", bufs=1))
        vp_ = ctx.enter_context(tc.tile_pool(name="vp", bufs=1))
        # ep: cross-head pipeline holds a constant ~16 exp tiles live
        ep = ctx.enter_context(tc.tile_pool(name="ep", bufs=18))
        op_ = ctx.enter_context(tc.tile_pool(name="op", bufs=4))
        opp = ctx.enter_context(tc.tile_pool(name="opp", bufs=4))
        rp = ctx.enter_context(tc.tile_pool(name="rp", bufs=2))
        outs_ = ctx.enter_context(tc.tile_pool(name="outs", bufs=3))
        # PSUM (8 banks): pm 2x[128,1024]=4 for scores/proj/v/oproj pieces;
        # po 4x[65,512]=4 for the per-head PV pair + normalize broadcast pair
        pm = ctx.enter_context(tc.tile_pool(name="pm", bufs=2, space="PSUM"))
        po_ = ctx.enter_context(tc.tile_pool(name="po", bufs=4, space="PSUM"))

        # ---- persistent weights (DMA order matches consumption order) ----
        Wk_sb = wW.tile([P, NM * DG], XDT, tag="W", name="Wk_sb")
        Wq_sb = wW.tile([P, NM * DG], XDT, tag="W", name="Wq_sb")
        Wv_sb = wW.tile([P, NM * DG], XDT, tag="W", name="Wv_sb")
        # Wo in natural [dv, dm] chunk layout, bf16 (output projection is bf16)
        Wo_sb = wp.tile([P, ND * DM], BF, tag="Wo")
        bq_sb = wp.tile([P, ND], FP, tag="bq")
        bk_sb = wp.tile([P, ND], FP, tag="bk")
        for m in range(NM):
            nc.sync.dma_start(Wk_sb[:, m * DG:(m + 1) * DG], io["Wk"][m * P:(m + 1) * P, :])
        for t in range(ND):
            nc.sync.dma_start(bq_sb[:, t:t + 1], io["bq"][t * P:(t + 1) * P].unsqueeze(1))
            nc.sync.dma_start(bk_sb[:, t:t + 1], io["bk"][t * P:(t + 1) * P].unsqueeze(1))
        # ones row lives at partition 64 so the broadcast matmul's lhsT shares
        # a base partition with the reciprocal row (matmul requires equal bases)
        ones_sb = wp.tile([DK + 1, DK], FP, tag="ones")
        nc.vector.memset(ones_sb[DK:DK + 1, :], 1.0)

        qT = [qk.tile([P, S], XDT, tag=f"qT{d}", name=f"qT{d}") for d in range(ND)]
        kT = [qk.tile([P, S], XDT, tag=f"kT{d}", name=f"kT{d}") for d in range(ND)]
        vps = [vp_.tile([P, HC * (DK + 1)], EXPDT, tag=f"v{st}", name=f"v{st}")
               for st in range(NST)]

        def stage_x(XT, c, eng):
            """stage one 512-column chunk of an X^T input (8 DMAs)"""
            xts = []
            for m in range(NM):
                xt = xs.tile([P, SC], XDT, tag="xs", name=f"xs{m}")
                eng.dma_start(xt[:], XT[m * P:(m + 1) * P, c * SC:(c + 1) * SC])
                xts.append(xt)
            return xts

        def proj_group(xts, c, d, Wsb, bsb, dst):
            """one 8-matmul projection accumulation group as a callable"""
            def fn():
                ps = pm.tile([P, SC], FP, tag="mm", name="psproj")
                for m in range(NM):
                    nc.tensor.matmul(
                        ps[:],
                        R(Wsb[:, m * DG + d * P: m * DG + (d + 1) * P]),
                        R(xts[m][:]),
                        start=(m == 0), stop=(m == NM - 1))
                # psum -> persistent sbuf, bias add fused (per-partition)
                nc.vector.tensor_scalar_add(
                    dst[d][:, c * SC:(c + 1) * SC], ps[:], bsb[:, d:d + 1])
            return fn

        def proj_qk(XT, Wsb, bsb, dst, cs, eng):
            """dst[d][p, s] = sum_m W[m, d*128+p] * X[s, m] + b"""
            for c in cs:
                xts = stage_x(XT, c, eng)
                for d in range(ND):
                    proj_group(xts, c, d, Wsb, bsb, dst)()

        def xv_dma(q):
            """stage XvT columns for sk-tiles 4q..4q+3"""
            xts = []
            for m in range(NM):
                xt = xv.tile([P, SC], XDT, tag="xv", name=f"xv{m}")
                nc.sync.dma_start(xt[:], io["XvT"][m * P:(m + 1) * P, q * SC:(q + 1) * SC])
                xts.append(xt)
            return xts

        def vproj_st(st, xts):
            si = st % 4
            ps = pm.tile([P, DG], FP, tag="mm", name="psv")
            for m in range(NM):
                nc.tensor.matmul(
                    ps[:],
                    R(xts[m][:, si * P:(si + 1) * P]),
                    R(Wv_sb[:, m * DG:(m + 1) * DG]),
                    start=(m == 0), stop=(m == NM - 1))
            v3o = vps[st][:].rearrange("p (h e) -> p h e", e=DK + 1)
            nc.vector.tensor_copy(v3o[:, :, 0:DK], ps[:].rearrange("p (h e) -> p h e", e=DK))
            nc.vector.memset(v3o[:, :, DK:DK + 1], 1.0)

        def scores_st(c2, h, st):
            d, po = divmod(h, 2)
            po *= DK
            ps_s = pm.tile([P, SC2], FP, tag="mm", name="pss")
            for q2 in range(SC2 // SC):
                nc.tensor.matmul(
                    ps_s[:, q2 * SC:(q2 + 1) * SC],
                    R(kT[d][po:po + DK, st * P:(st + 1) * P]),
                    R(qT[d][po:po + DK, c2 * SC2 + q2 * SC: c2 * SC2 + (q2 + 1) * SC]),
                    start=True, stop=True)
            et = ep.tile([P, SC2], EXPDT, tag="ep", name="et")
            nc.scalar.activation(et[:], ps_s[:], AF.Exp, scale=_SCALE)
            return et

        def emit_pv(psos, h, ets, st):
            for q2 in range(SC2 // SC):
                nc.tensor.matmul(
                    psos[q2][:],
                    vps[st][:, h * (DK + 1):(h + 1) * (DK + 1)],
                    ets[st][:, q2 * SC:(q2 + 1) * SC],
                    start=(st == 0), stop=(st == NST - 1))

        def alloc_psos():
            return [po_.tile([DK + 1, SC], FP, tag="ov", name="pso")
                    for _ in range(SC2 // SC)]

        opairs = {}  # (c2, d) -> [128, SC2] bf16 tile holding two heads' oT

        def norm(c2, h, psos):
            """o = oT_unnorm * (1/sums); 1/sums broadcast across dv via a tiny
            PE matmul. nrm rows 0-63 hold the broadcast, row 64 the recip.
            The normalized head is then DMA-repacked (partition move) into a
            two-head pair tile so the output projection can contract K=128."""
            nrm = rp.tile([DK + 1, SC2], FP, tag="nrm", name="nrm")
            ot = op_.tile([DK, SC2], BF, tag="o", name=f"ot{h}")
            psbs = []
            for q2 in range(SC2 // SC):
                cs = slice(q2 * SC, (q2 + 1) * SC)
                nc.vector.reciprocal(nrm[DK:DK + 1, cs], psos[q2][DK:DK + 1, :])
                psb = po_.tile([DK, SC], FP, tag="ov", name="psb")
                nc.tensor.matmul(psb[:], R(ones_sb[DK:DK + 1, :]),
                                 R(nrm[DK:DK + 1, cs]), start=True, stop=True)
                psbs.append(psb)
            for q2 in range(SC2 // SC):
                cs = slice(q2 * SC, (q2 + 1) * SC)
                nc.vector.tensor_copy(nrm[0:DK, cs], psbs[q2][:])
                nc.vector.tensor_mul(ot[:, cs], psos[q2][0:DK, :], nrm[0:DK, cs])
            d, po = divmod(h, 2)
            if po == 0:
                opairs[(c2, d)] = opp.tile([P, SC2], BF, tag="opair", name=f"op{d}")
            # scalar-ring DMA: the only engine class that can move data across
            # partitions (SBUF->SBUF)
            nc.scalar.dma_start(opairs[(c2, d)][po * DK:(po + 1) * DK, :], ot[:])

        ets = {}   # (c2, h) -> {st: exp tile}
        ots = {}   # (c2, h) -> normalized oT tile

        def oproj_pieces(c2):
            """16 (j, dmc) output-projection pieces for one chunk, bf16"""
            fns = []
            ostg_box = {}

            def piece(j, dmc):
                def fn():
                    if dmc == 0:
                        ostg_box[j] = outs_.tile([P, DM], FP, tag="os", name="ostg")
                    ostg = ostg_box[j]
                    ps2 = pm.tile([P, SC], FP, tag="mm", name="psout")
                    for d in range(ND):
                        nc.tensor.matmul(
                            ps2[:],
                            opairs[(c2, d)][:, j * P:(j + 1) * P],
                            Wo_sb[:, d * DM + dmc * SC: d * DM + (dmc + 1) * SC],
                            start=(d == 0), stop=(d == ND - 1))
                    if dmc % 2 == 0:
                        nc.vector.tensor_copy(ostg[:, dmc * SC:(dmc + 1) * SC], ps2[:])
                    else:
                        nc.scalar.copy(ostg[:, dmc * SC:(dmc + 1) * SC], ps2[:])
                    if dmc == DM // SC - 1:
                        # chunk 0 outputs ride gpsimd SWDGE (sync ring is
                        # still loading inputs); chunk 1's ride the by-then
                        # idle sync HWDGE ring (lower latency for the tail)
                        eng = nc.gpsimd if c2 == 0 else nc.sync
                        eng.dma_start(
                            io["out"][(c2 * (SC2 // P) + j) * P:(c2 * (SC2 // P) + j + 1) * P, :],
                            ostg[:])
                return fn
            for j in range(SC2 // P):
                for dmc in range(DM // SC):
                    fns.append(piece(j, dmc))
            return fns

        def block(cur, prev, pieces=()):
            """one pipeline block: PV of `prev` head + scores/exp of `cur`,
            with extra PE work `pieces` spread across the 16 sk-steps.
            pieces: list (spread evenly) or dict {st: [fns]} (explicit)."""
            psos = alloc_psos() if prev else None
            e_cur = {}
            for st in range(NST):
                if isinstance(pieces, dict):
                    todo = pieces.get(st, ())
                else:
                    todo = pieces[st * len(pieces) // NST:(st + 1) * len(pieces) // NST]
                for fn in todo:
                    fn()
                if prev:
                    emit_pv(psos, prev[1], ets[prev], st)
                if cur:
                    e_cur[st] = scores_st(cur[0], cur[1], st)
            if cur:
                ets[cur] = e_cur
            if prev:
                norm(prev[0], prev[1], psos)
                del ets[prev]

        # ---------------- flow ----------------
        # Emission order IS the per-engine stream order: the exp stream must
        # start early and never starve, so everything else (v/q projections,
        # output projections) is diced into pieces and interleaved. K/V loads
        # ride the sync HWDGE ring; Q loads ride the scalar HWDGE ring so the
        # two input streams don't serialize behind each other.
        for m in range(NM):
            nc.scalar.dma_start(Wq_sb[:, m * DG:(m + 1) * DG], io["Wq"][m * P:(m + 1) * P, :])
        proj_qk(io["XkT"], Wk_sb, bk_sb, kT, (0,), nc.sync)
        proj_qk(io["XqT"], Wq_sb, bq_sb, qT, (0, 1), nc.scalar)
        # k chunks 1-3 ride inside block (0,0): scores(0,0,st) only needs the
        # k chunk st//4 (d-tile 0), so first exp waits only on k chunk 0.
        # c3's staging is itself a piece (xs pool has 16 slots; c1+c2 fill it)
        kstage = {c: stage_x(io["XkT"], c, nc.sync) for c in (1, 2)}

        def stage_k3():
            kstage[3] = stage_x(io["XkT"], 3, nc.sync)

        def k3_group(d):
            def fn():
                proj_group(kstage[3], 3, d, Wk_sb, bk_sb, kT)()
            return fn

        kpieces = {1: [proj_group(kstage[1], 1, 0, Wk_sb, bk_sb, kT)],
                   2: [proj_group(kstage[1], 1, 1, Wk_sb, bk_sb, kT)],
                   3: [proj_group(kstage[2], 2, 0, Wk_sb, bk_sb, kT), stage_k3],
                   5: [proj_group(kstage[2], 2, 1, Wk_sb, bk_sb, kT)],
                   9: [k3_group(0)],
                   12: [k3_group(1)]}
        for m in range(NM):
            nc.sync.dma_start(Wv_sb[:, m * DG:(m + 1) * DG], io["Wv"][m * P:(m + 1) * P, :])
        xvq = {0: xv_dma(0)}

        block(cur=(0, 0), prev=None, pieces=kpieces)

        # vproj block: v-projection + PV(0,0) + scores(0,1)
        psos = alloc_psos()
        e_cur = {}
        for st in range(NST):
            q = st // 4
            if st % 4 == 0 and q < 3:
                xvq[q + 1] = xv_dma(q + 1)
            vproj_st(st, xvq[q])
            emit_pv(psos, 0, ets[(0, 0)], st)
            e_cur[st] = scores_st(0, 1, st)
        ets[(0, 1)] = e_cur
        norm(0, 0, psos)
        del ets[(0, 0)]

        # q half-1 projection rides inside the (0,2) block
        qh1 = []
        for c in (2, 3):
            xts_q1 = stage_x(io["XqT"], c, nc.scalar)
            for d in range(ND):
                qh1.append(proj_group(xts_q1, c, d, Wq_sb, bq_sb, qT))
        block(cur=(0, 2), prev=(0, 1), pieces=qh1)
        block(cur=(0, 3), prev=(0, 2))
        for c in range(ND):
            nc.sync.dma_start(Wo_sb[:, c * DM:(c + 1) * DM], io["Wo"][c * P:(c + 1) * P, :])
        block(cur=(1, 0), prev=(0, 3))
        block(cur=(1, 1), prev=(1, 0), pieces=oproj_pieces(0))
        block(cur=(1, 2), prev=(1, 1))
        block(cur=(1, 3), prev=(1, 2))
        block(cur=None, prev=(1, 3))
        for fn in oproj_pieces(1):
            fn()


def build_nc(mm_mode=None):
    """Build + compile the single-core SPMD Bass program."""
    import concourse.bacc as bacc
    import concourse.mybir as mybir
    import concourse.tile as tile

    mm_mode = mm_mode or MM_MODE
    FP = mybir.dt.float32
    BF = mybir.dt.bfloat16
    XDT = FP if mm_mode == "fp32r" else BF

    nc = bacc.Bacc("TRN2", target_bir_lowering=False, debug=False,
                   enable_asserts=False)
    io = {
        "XqT": nc.declare_dram_parameter("XqT", [_DM, _S], XDT, isOutput=False),
        "XkT": nc.declare_dram_parameter("XkT", [_DM, _S], XDT, isOutput=False),
        "XvT": nc.declare_dram_parameter("XvT", [_DM, _S], XDT, isOutput=False),
        "Wq": nc.declare_dram_parameter("Wq", [_DM, _DG], XDT, isOutput=False),
        "Wk": nc.declare_dram_parameter("Wk", [_DM, _DG], XDT, isOutput=False),
        "Wv": nc.declare_dram_parameter("Wv", [_DM, _DG], XDT, isOutput=False),
        # Wo is consumed in bf16 regardless of mode (output projection runs bf16)
        "Wo": nc.declare_dram_parameter("Wo", [_DG, _DM], BF, isOutput=False),
        "bq": nc.declare_dram_parameter("bq", [_DG], FP, isOutput=False),
        "bk": nc.declare_dram_parameter("bk", [_DG], FP, isOutput=False),
        "out": nc.declare_dram_parameter("out", [_S, _DM], FP, isOutput=True),
    }
    with tile.TileContext(nc) as tc:
        _emit(nc, tc, io, mm_mode)
    nc.compile()
    return nc


def make_in_maps(Q, K, V, Wq, bq, Wk, bk, Wv, Wo, mm_mode=None):
    mm_mode = mm_mode or MM_MODE
    import ml_dtypes
    xdt = np.float32 if mm_mode == "fp32r" else ml_dtypes.bfloat16
    f32 = lambda a: np.ascontiguousarray(np.asarray(a, dtype=np.float32))
    cvt = lambda a: np.ascontiguousarray(np.asarray(a, dtype=np.float32).astype(xdt))
    bf = lambda a: np.ascontiguousarray(
        np.asarray(a, dtype=np.float32).astype(ml_dtypes.bfloat16))
    Q, K, V = f32(Q), f32(K), f32(V)
    in_maps = []
    for b in range(_B):
        XqT = cvt(Q[b].T)
        XkT = cvt(K[b].T)
        XvT = cvt(V[b].T)
        for g in range(_GROUPS):
            sl = slice(g * _DG, (g + 1) * _DG)
            in_maps.append({
                "XqT": XqT, "XkT": XkT, "XvT": XvT,
                "Wq": cvt(np.asarray(Wq)[:, sl]),
                "Wk": cvt(np.asarray(Wk)[:, sl]),
                "Wv": cvt(np.asarray(Wv)[:, sl]),
                "Wo": bf(np.asarray(Wo)[sl, :]),
                "bq": f32(np.asarray(bq)[sl]),
                "bk": f32(np.asarray(bk)[sl]),
            })
    return in_maps


def _ensure_profile_hook_importable():
    """Some containers lack antenv.axon_hooks; stub it so trace=True degrades
    to an untraced run instead of crashing."""
    import sys as _sys
    import types as _types
    try:
        import antenv.axon_hooks  # noqa: F401
    except Exception:
        m = _types.ModuleType("antenv.axon_hooks")
        m.get_axon_ntff_profile_hook = lambda: None
        _sys.modules["antenv.axon_hooks"] = m


def kernel(Q, K, V, Wq, bq, Wk, bk, Wv, bv, Wo, bo):
    from concourse.bass_utils import run_bass_kernel_spmd
    if PROFILE:
        _ensure_profile_hook_importable()

    key = ("nc", MM_MODE)
    if key not in _CACHE:
        _CACHE[key] = build_nc(MM_MODE)
    nc = _CACHE[key]

    in_maps = make_in_maps(Q, K, V, Wq, bq, Wk, bk, Wv, Wo, MM_MODE)
    res = run_bass_kernel_spmd(nc, in_maps, list(range(_B * _GROUPS)),
                               trace=PROFILE)
    LAST_EXEC_NS["ns"] = res.exec_time_ns
    LAST_EXEC_NS["result"] = res
    outs = [r["out"] for r in res.results]
    full = np.stack([sum(outs[b * _GROUPS:(b + 1) * _GROUPS]) for b in range(_B)])
    # exact host-side fold of the v-projection bias and output bias:
    # att rows sum to 1, so att @ (v + 1*bv) @ Wo + bo = device_out + bv@Wo + bo
    fold = (np.asarray(bv, np.float32) @ np.asarray(Wo, np.float32)
            + np.asarray(bo, np.float32))
    full = full + fold[None, None, :]
    return full.astype(np.float32)

